# revision 1
# baseline (speedup 1.0000x reference)
"""GCGRUCell Trainium2 kernel — 8-core SPMD, fp8 aggregation path.

Math (per reference):
  value = sigmoid(cat([x, h]) @ W_fc + b_fc);  r, u = split(value)
  X0 = cat([x, r*h])                                (B, N, D)   D=66
  Y  = X0 @ Wg_odd                                  (B, N, U)
  X1 = S @ Y          (segment_sum over E edges, commuted with Wg_odd)
  c  = tanh(X0 @ Wg_even + X1 + b_g)
  out = u * hx + (1-u) * c

Sharding: nodes 1250/core; within a core, nodes are greedily packed
into 10 blocks of 128 so every (core, block) has <= 2048 incoming
edges (uniform eb=2048).  Phase A per block/half: r logits + one
sigmoid, u via transposing matmuls, r*h written in place into catT
(which persists in SBUF for all 10 blocks), y-only matmuls with a
contiguous PSUM->fp8 copy.  Y rows are all-gathered (5 pieces,
overlapping A) into a (10240, 1024) fp8 table.  The one-hot tables
and hx are prefetched into SBUF during phase A via the idle Pool
engine.  Phase B gathers 4096 edge-source rows per 2-block group
(dma_gather, 1KB fp8 rows) and scatter-sums them with one-hot fp8
DoubleRow matmuls (256 edges per PE instruction, fp32 PSUM); the Z0
term is accumulated into the same PSUM directly from the persisted
catT tiles (16 bf16 matmuls per block), so Z0 never exists in SBUF.
Phase C: tanh on Act straight from PSUM, then bf16 sub/mul/add on
DVE; out is written bf16 and upcast on the host.

Row order: tile t in [0,16) = batch, 128 nodes per tile; per-block Y
writes and out writes are single contiguous DMAs.  Feature order:
[h (0:64), xi (64:66), ones (66)].  Biases ride the ones row.
"""

from contextlib import nullcontext

import numpy as np
import ml_dtypes
import concourse.bass as bass
import concourse.bacc as bacc
import concourse.mybir as mybir
import concourse.tile as tile
from bass_rust import add_dep_helper
from concourse.bass_utils import run_bass_kernel_spmd

F32 = mybir.dt.float32
BF16 = mybir.dt.bfloat16
FP8 = mybir.dt.float8e4
I16 = mybir.dt.int16

NCORES = 8
N, B, U, DIN = 10000, 16, 64, 2
D = DIN + U                      # 66
K = D + 1                        # 67 (ones row for biases)
KP = 72                          # K padded to a multiple of 8: DMAs with
                                 # non-8-aligned partition counts run ~10x
                                 # slower (measured 22GB/s at 67 parts vs
                                 # 170+GB/s at 64/72/128)
NPC = N // NCORES                # 1250 nodes/core
NPC_PAD = 1280                   # 10 blocks of 128
NBLK = NPC_PAD // 128            # 10 dest blocks
ROWS = B * NPC_PAD               # 20480
CHUNK_ROWS = 16 * 128            # one block of nodes x all batches
W = B * U                        # 1024 gather row width
AGS = 5                          # allgather split count
AG_BOUNDS = [round(i * NBLK / AGS) for i in range(AGS + 1)]
GRP = 2                          # dest blocks per gather


def _ytab_row(c, nl):
    """y_tab row index of node (core c, local slot nl) after the split
    AllGather: piece p's output is [rank0 blocks j0:j1, rank1 blocks
    j0:j1, ...] concatenated."""
    j = nl // 128
    si = nl % 128
    bounds = np.asarray(AG_BOUNDS)
    p = np.searchsorted(bounds, j, side="right") - 1
    j0 = bounds[p]
    j1 = bounds[p + 1]
    return (j0 * NCORES + c * (j1 - j0) + (j - j0)) * 128 + si


def build_kernel(eb: tuple[int, ...], stage: int = 5,
                 r_a: int = 1, r_ag: int = 1, r_bc: int = 1):
    """eb[j] = padded edge count of dest block j (same for all cores),
    multiple of 256.
    stage: 1=phase A only, 2=+allgather, 3=+gathers, 4=full
    r_a/r_ag/r_bc: repetition counts (hardware For_i loops / replicated
    collective) for wall-clock-difference timing; correctness needs 1."""
    assert len(eb) == NBLK and all(e % 256 == 0 for e in eb)
    dkt_tot = sum(e // 256 for e in eb)

    nc = bacc.Bacc("TRN2", target_bir_lowering=False, debug=False,
                   num_devices=NCORES, num_swdge_queues=2)

    # ---- I/O ----
    catT_in = nc.dram_tensor("catT", [KP, ROWS], BF16, kind="ExternalInput")
    hxb_in = nc.dram_tensor("hx_blk", [128, NBLK * W], BF16,
                            kind="ExternalInput")
    wfc_in = nc.dram_tensor("wfc", [KP, 128], BF16, kind="ExternalInput")
    wg_in = nc.dram_tensor("wg", [KP, 128], BF16, kind="ExternalInput")
    idx_in = nc.dram_tensor("idxw", [128, sum(eb) // 16], I16,
                            kind="ExternalInput")
    oh_in = nc.dram_tensor("oh", [128, dkt_tot * 256], FP8,
                           kind="ExternalInput")
    out_dram = nc.dram_tensor("out", [NBLK, 128, B, U], BF16,
                              kind="ExternalOutput")

    # ---- internal DRAM (collective) ----
    y_loc = nc.dram_tensor("y_loc", [NBLK * 128, W], FP8, kind="Internal")
    y_tab = nc.dram_tensor("y_tab", [NBLK * NCORES * 128, W], FP8,
                           kind="Internal", addr_space="Shared")

    with tile.TileContext(nc) as tc:
        with (
            tc.tile_pool(name="persist", bufs=1) as pp,
            tc.tile_pool(name="pa", bufs=1) as pa,
        ):
            wfc = pp.tile([KP, 128], BF16)
            nc.sync.dma_start(wfc[:], wfc_in[:])
            wg = pp.tile([KP, 128], BF16)
            nc.sync.dma_start(wg[:], wg_in[:])
            usb = pp.tile([128, NBLK * W], BF16)    # 20KB/part
            # prefetched for phase B/C on the idle Pool engine (the SP
            # ring carries phase A's catT loads)
            oh_sb = pp.tile([128, dkt_tot * 256], FP8)
            nc.gpsimd.dma_start(oh_sb[:], oh_in[:])
            hx_sb = pp.tile([128, NBLK * W], BF16)
            nc.gpsimd.dma_start(hx_sb[:], hxb_in[:])

            # ================= PHASE A =================
            # One big catT load + one persistent Y staging buffer: DMA
            # issue on the HWDGE rings costs ~5us serial apiece, so phase
            # A uses 1 catT DMA + 5 per-AG-piece y_loc writes instead of
            # 10 + 10 per-block ones.
            y_writes = []
            catT = pa.tile([KP, ROWS], BF16)
            ystage = pa.tile([128, NBLK * W], FP8)
            with (
                tc.tile_pool(name="pa_sig", bufs=2) as psig,
                tc.tile_pool(name="ps_r", bufs=2, space="PSUM") as ps_r,
                tc.tile_pool(name="ps_u", bufs=2, space="PSUM") as ps_u,
                tc.tile_pool(name="ps_y", bufs=2, space="PSUM") as ps_y,
                tc.For_i(0, r_a, 1) if r_a > 1 else nullcontext(),
            ):
                nc.sync.dma_start(catT[:], catT_in[:])
                for blk in range(NBLK):
                    boff = blk * CHUNK_ROWS
                    for half in range(2):
                        hsl = slice(boff + half * 1024, boff + (half + 1) * 1024)
                        pr = ps_r.tile([U, 1024], F32)
                        for g in range(2):
                            sl = slice(boff + (half * 2 + g) * 512,
                                       boff + (half * 2 + g + 1) * 512)
                            nc.tensor.matmul(
                                pr[:, g * 512:(g + 1) * 512],
                                wfc[:, 0:U], catT[:, sl],
                                start=True, stop=True)
                        sig = psig.tile([U, 1024], BF16, tag="sig")
                        nc.scalar.activation(
                            sig[:], pr[:],
                            mybir.ActivationFunctionType.Sigmoid)
                        # u-logits: 8 transposing tiles share one PSUM bank
                        pu = ps_u.tile([128, 512], F32)
                        for i in range(8):
                            b = half * 8 + i
                            tsl = slice(boff + b * 128, boff + (b + 1) * 128)
                            nc.tensor.matmul(
                                pu[:, i * U:(i + 1) * U],
                                catT[:, tsl], wfc[:, U:128],
                                start=True, stop=True)
                        # r*h in place (u-matmuls above read pre-mul catT)
                        nc.vector.tensor_mul(
                            catT[0:U, hsl], sig[:], catT[0:U, hsl])
                        nc.scalar.activation(
                            usb[:, blk * W + half * 512:
                                 blk * W + (half + 1) * 512],
                            pu[:], mybir.ActivationFunctionType.Sigmoid)
                        # y: 8 tiles x [node, y] share one PSUM bank
                        pyy = ps_y.tile([128, 512], F32)
                        for i in range(8):
                            b = half * 8 + i
                            tsl = slice(boff + b * 128, boff + (b + 1) * 128)
                            nc.tensor.matmul(
                                pyy[:, i * U:(i + 1) * U],
                                catT[:, tsl], wg[:, 0:U],
                                start=True, stop=True)
                        nc.vector.tensor_copy(
                            ystage[:, blk * W + half * 512:
                                   blk * W + (half + 1) * 512], pyy[:])
                    if blk + 1 in AG_BOUNDS[1:]:
                        p = AG_BOUNDS.index(blk + 1) - 1
                        j0, j1 = AG_BOUNDS[p], AG_BOUNDS[p + 1]
                        ydma = nc.scalar.dma_start(
                            y_loc[j0 * 128: j1 * 128, :]
                            .rearrange("(j n) w -> n j w", n=128),
                            ystage[:, j0 * W: j1 * W]
                            .rearrange("n (j w) -> n j w", w=W))
                        y_writes.append(ydma)

            # ================= ALLGATHER (split, overlaps phase A) ======
            ccs = []
            if stage >= 2:
                prev_cc = None
                for rep in range(r_ag):
                    for p in range(AGS):
                        j0, j1 = AG_BOUNDS[p], AG_BOUNDS[p + 1]
                        cc = nc.gpsimd.collective_compute(
                            "AllGather", mybir.AluOpType.bypass,
                            replica_groups=[list(range(NCORES))],
                            ins=[y_loc[j0 * 128: j1 * 128, :]],
                            outs=[y_tab[j0 * NCORES * 128:
                                        j1 * NCORES * 128, :]],
                        )
                        ccs.append(cc)
                        if r_ag > 1 and prev_cc is not None:
                            add_dep_helper(cc.ins, prev_cc.ins, sync=True,
                                           reason="serialize ag reps")
                        prev_cc = cc
                        if r_a == 1 and r_ag == 1:
                            add_dep_helper(cc.ins, y_writes[p].ins,
                                           sync=True,
                                           reason="allgather reads y_loc")

            # ================= PHASE B + C =================
            with (
                tc.tile_pool(name="pg", bufs=2) as pg,
                tc.tile_pool(name="pi", bufs=NBLK // GRP) as pi,
                tc.tile_pool(name="pc", bufs=2) as pcl,
                tc.tile_pool(name="ps_b", bufs=2, space="PSUM") as ps_b,
                tc.For_i(0, r_bc, 1) if r_bc > 1 else nullcontext(),
            ):
                for m in range(NBLK // GRP if stage >= 3 else 0):
                    ne_g = sum(eb[m * GRP: (m + 1) * GRP])
                    idx_off = sum(eb[:m * GRP]) // 16
                    idxt = pi.tile([128, ne_g // 16], I16, tag="idx")
                    nc.scalar.dma_start(
                        idxt[:],
                        idx_in[:, idx_off: idx_off + ne_g // 16])
                    gt = pg.tile([128, ne_g // 128, W], FP8, tag="G")
                    gather = nc.gpsimd.dma_gather(
                        out_ap=gt[:],
                        in_ap=y_tab[:],
                        idxs_ap=idxt[:],
                        num_idxs=ne_g,
                        num_idxs_reg=ne_g,
                        elem_size=W,
                        single_packet=False,
                        queue_num=m % 2,
                    )
                    if r_bc == 1 and r_ag == 1:
                        for cc in ccs:
                            add_dep_helper(
                                gather.ins, cc.ins, sync=True,
                                reason="gather reads allgathered y_tab")
                    if stage < 4:
                        continue
                    outg = pcl.tile([128, GRP, W], BF16, tag="outg")
                    for jj in range(GRP):
                        j = m * GRP + jj
                        ndkt = eb[j] // 256
                        dkt_off = sum(eb[:j]) // 256
                        goff = sum(eb[m * GRP: j]) // 128
                        px1 = ps_b.tile([128, W], F32)
                        for d in range(ndkt):
                            ot = oh_sb[:, (dkt_off + d) * 256:
                                       (dkt_off + d + 1) * 256] \
                                .rearrange("p (k f) -> p k f", k=2)
                            first = d == 0
                            gsl = slice(goff + 2 * d, goff + 2 * d + 2)
                            nc.tensor.matmul(
                                px1[:, 0:512], ot, gt[:, gsl, 0:512],
                                start=first, stop=False,
                                perf_mode=mybir.MatmulPerfMode.DoubleRow,
                                skip_group_check=True)
                            nc.tensor.matmul(
                                px1[:, 512:1024], ot, gt[:, gsl, 512:1024],
                                start=first, stop=False,
                                perf_mode=mybir.MatmulPerfMode.DoubleRow,
                                skip_group_check=True)
                        # Z0 joins the same accumulation straight from the
                        # persisted (post r*h) catT columns
                        for b in range(B):
                            tsl = slice(j * CHUNK_ROWS + b * 128,
                                        j * CHUNK_ROWS + (b + 1) * 128)
                            nc.tensor.matmul(
                                px1[:, b * U:(b + 1) * U],
                                catT[:, tsl], wg[:, U:128],
                                start=False, stop=True,
                                skip_group_check=True)
                        # ---- phase C for block j ----
                        ct = pcl.tile([128, W], BF16, tag="c")
                        nc.scalar.activation(
                            ct[:], px1[:],
                            mybir.ActivationFunctionType.Tanh)
                        dt_ = pcl.tile([128, W], BF16, tag="d")
                        nc.vector.tensor_sub(
                            dt_[:], hx_sb[:, j * W:(j + 1) * W], ct[:])
                        nc.vector.tensor_mul(dt_[:], dt_[:],
                                             usb[:, j * W:(j + 1) * W])
                        nc.vector.tensor_add(outg[:, jj, :], dt_[:], ct[:])
                    nc.sync.dma_start(
                        out_dram[m * GRP:(m + 1) * GRP]
                        .rearrange("j n b u -> n j (b u)"),
                        outg[:])

    nc.compile()
    return nc


# ---------------- host side ----------------

def prep_inputs(inputs, hx, rows, cols, vals, W_fc, b_fc, W_g, b_g):
    """Build the 8 per-core input maps + the edge-block geometry."""
    xi = np.asarray(inputs).reshape(B, N, DIN)
    h = np.asarray(hx).reshape(B, N, U)
    rows = np.asarray(rows); cols = np.asarray(cols); vals = np.asarray(vals)

    core_of = rows // NPC
    # ---- per-core node->block balancing (target <=2048 edges/block) ----
    perms = []          # perms[k][slot] = original local node (or -1 pad)
    slot_of = np.full((NCORES, NPC), -1, np.int64)  # local node -> slot
    for k in range(NCORES):
        deg = np.bincount(rows[core_of == k] - k * NPC, minlength=NPC)
        order = np.argsort(-deg, kind="stable")
        blk_edges = np.zeros(NBLK, np.int64)
        blk_nodes = [[] for _ in range(NBLK)]
        for n in order:
            best, be = -1, 1 << 60
            for j in range(NBLK):
                if len(blk_nodes[j]) < 128 and blk_edges[j] < be:
                    best, be = j, blk_edges[j]
            blk_nodes[best].append(n)
            blk_edges[best] += deg[n]
        perm = np.full(NPC_PAD, -1, np.int64)
        for j in range(NBLK):
            nodes = blk_nodes[j]
            perm[j * 128: j * 128 + len(nodes)] = nodes
            for si, n in enumerate(nodes):
                slot_of[k, n] = j * 128 + si
        perms.append(perm)

    per_core = []           # (cols, dest_slot, val) per core per block
    counts = np.zeros((NCORES, NBLK), np.int64)
    for k in range(NCORES):
        m = core_of == k
        slot = slot_of[k, rows[m] - k * NPC]
        c_l = cols[m]
        v_l = vals[m]
        blk = slot // 128
        per_block = []
        for j in range(NBLK):
            bm = blk == j
            per_block.append((c_l[bm], slot[bm] % 128, v_l[bm]))
            counts[k, j] = bm.sum()
        per_core.append(per_block)

    eb = tuple(max(256, int(-(-counts[:, j].max() // 256) * 256))
               for j in range(NBLK))
    dkt_tot = sum(e // 256 for e in eb)

    # source (global node) -> y_tab row, via the owning core's slot
    src_core = np.arange(N) // NPC
    src_slot = slot_of[src_core, np.arange(N) % NPC]
    ytab_of_node = _ytab_row(src_core, src_slot).astype(np.int16)

    # feature order everywhere: [h (0:64), xi (64:66), ones (66)]
    perm_f = np.concatenate([np.arange(DIN, D), np.arange(DIN)])
    wfc_ext = np.zeros((KP, 128), np.float32)
    wfc_ext[:D] = np.asarray(W_fc)[perm_f]
    wfc_ext[D] = np.asarray(b_fc)
    wg = np.asarray(W_g).reshape(D, 2, U)
    wg_comb = np.zeros((KP, 128), np.float32)
    wg_comb[:D, :U] = wg[perm_f, 1, :]       # odd rows -> Y
    wg_comb[:D, U:] = wg[perm_f, 0, :]       # even rows -> Z0
    wg_comb[D, U:] = np.asarray(b_g)         # b_g into Z0

    in_maps = []
    for k in range(NCORES):
        sl = slice(k * NPC, (k + 1) * NPC)
        perm = perms[k]
        valid = perm >= 0
        xi_p = np.zeros((B, NPC_PAD, DIN), np.float32)
        xi_p[:, valid] = xi[:, sl][:, perm[valid]]
        h_p = np.zeros((B, NPC_PAD, U), np.float32)
        h_p[:, valid] = h[:, sl][:, perm[valid]]
        # rows ordered (blk, b, nl): tile t = blk*16 + b
        catT = np.zeros((KP, ROWS), np.float32)
        catT[0:U] = (h_p.reshape(B, NBLK, 128, U)
                     .transpose(3, 1, 0, 2).reshape(U, ROWS))
        catT[U:D] = (xi_p.reshape(B, NBLK, 128, DIN)
                     .transpose(3, 1, 0, 2).reshape(DIN, ROWS))
        catT[D] = 1.0
        hx_blk = (h_p.reshape(B, NBLK, 128, U)
                  .transpose(2, 1, 0, 3).reshape(128, NBLK * B * U))

        idx_w = np.zeros((128, sum(eb) // 16), np.int16)
        oh = np.zeros((128, dkt_tot * 256), ml_dtypes.float8_e4m3)
        ioff = 0
        dktoff = 0
        for j in range(NBLK):
            c_l, dl, v_l = per_core[k][j]
            ne = len(c_l)
            idx = np.zeros(eb[j], np.int16)
            idx[:ne] = ytab_of_node[c_l]
            wrap = idx.reshape(eb[j] // 16, 16).T        # (16, eb/16)
            idx_w[:, ioff: ioff + eb[j] // 16] = np.tile(wrap, (8, 1))
            e_pos = np.arange(ne)
            oh_cols = ((dktoff + e_pos // 256) * 2 +
                       (e_pos // 128) % 2) * 128 + dl
            oh[e_pos % 128, oh_cols] = v_l.astype(np.float32)
            ioff += eb[j] // 16
            dktoff += eb[j] // 256

        in_maps.append({
            "catT": catT.astype(ml_dtypes.bfloat16),
            "hx_blk": hx_blk.astype(ml_dtypes.bfloat16),
            "wfc": wfc_ext.astype(ml_dtypes.bfloat16),
            "wg": wg_comb.astype(ml_dtypes.bfloat16),
            "idxw": idx_w,
            "oh": oh,
        })
    return eb, in_maps, perms


_CACHE: dict = {}


def assemble_out(results, perms):
    """results[k]['out'] is (NBLK, 128, B, U) bf16; -> (B, N*U) f32."""
    full = np.empty((N, B, U), np.float32)
    for k in range(NCORES):
        o = results[k]["out"].reshape(NPC_PAD, B, U).astype(np.float32)
        perm = perms[k]
        valid = perm >= 0
        full[k * NPC + perm[valid]] = o[valid]
    return full.transpose(1, 0, 2).reshape(B, N * U)


def run(inputs, hx, rows, cols, vals, W_fc, b_fc, W_g, b_g):
    eb, in_maps, perms = prep_inputs(inputs, hx, rows, cols, vals,
                                     W_fc, b_fc, W_g, b_g)
    if eb not in _CACHE:
        _CACHE[eb] = build_kernel(eb)
    nc = _CACHE[eb]
    res = run_bass_kernel_spmd(nc, in_maps, core_ids=list(range(NCORES)))
    return assemble_out(res.results, perms)


def kernel(inputs, hx, rows, cols, vals, W_fc, b_fc, W_g, b_g):
    """Harness entry: full (unsharded) inputs -> full output (B, N*U)."""
    out = run(inputs, hx, rows, cols, vals, W_fc, b_fc, W_g, b_g)
    return out.astype(np.float32)

